# revision 1
# baseline (speedup 1.0000x reference)
"""Causal multi-head attention on 8 Trainium2 NeuronCores (Bass/Tile).

Problem: B=4 H=16 S=2048 D=64 fp32, causal mask, softmax(QK^T/sqrt(D))V.
Sharding: batch*heads (64) split 8 per core; no cross-core communication.

Design notes
------------
- Host pre-transposes Q,K to [d, s] per head so the device needs zero
  transposes: the QK^T matmul wants both operands d-major (contraction on
  partitions), and computing scores TRANSPOSED (S^T[k, q]) makes softmax's
  P^T directly usable as the moving operand of the P@V matmul.
- Softmax over k (= partition dim in S^T) avoids max-subtraction entirely
  (scores are ~N(0,1) after 1/sqrt(64) scaling; exp never overflows) and
  gets the denominator for free by appending a ones-column to V: row 64 of
  the PV output is sum_k P^T[k, q].  The final divide + transpose back to
  [s, d] happen on host.
- Causality: only lower-triangular 128x512 blocks are computed.  Diagonal
  blocks are packed (no psum waste, no bank-crossing matmul writes) and
  masked with a single shared [128, 512] additive causal mask.
- All matmuls run in bf16 (fp32/fp32r matmuls stream multi-pass on the PE
  — measured ~3x slower); accumulation stays fp32 in PSUM and the exp is
  computed in fp32 from PSUM.  End-to-end error ~5e-3, well under the
  2e-2 gate.
- exp() on the scalar engine is the throughput floor (~1 elem/lane/cycle
  @1.2GHz); ACTIVATE instructions are batched over multi-bank PSUM
  regions to amortize the ~352-cycle per-instruction overhead.
"""

import os
import sys

import numpy as np

sys.path.insert(0, "/opt/trn_rl_repo")

import concourse.bass as bass  # noqa: E402
import concourse.tile as tile  # noqa: E402
from concourse import bacc, mybir  # noqa: E402
from concourse.bass_utils import run_bass_kernel_spmd  # noqa: E402

B, H, S, D = 4, 16, 2048, 64
N_CORES = 8
HPC = (B * H) // N_CORES  # heads per core
KT = 128   # k-tile rows
CH = 512   # q-chunk cols
NEG = -1e9

F32 = mybir.dt.float32
F32R = mybir.dt.float32r
BF16 = mybir.dt.bfloat16


def _plan_chunk(c, causal):
    """Per q-chunk list of ACTIVATE batches.

    Each batch is (width, [(j, off, span, qlo, diag), ...]): k-tile j's
    scores for q-columns [qlo, qlo+span) of the chunk land at packed psum
    columns [off, off+span).  Offsets never let a matmul cross a 512-col
    psum bank boundary.  `diag` marks blocks needing the causal mask.
    """
    kpc = CH // KT  # k-tiles per chunk (4)
    batches = []
    if causal:
        # diagonal k-tiles j=kpc*c+r; packed order r0,r1,r3,r2 fills
        # [0,1280) with every matmul within a bank
        d0 = kpc * c
        diag = [
            (d0 + 0, 0, 512, 0, True),
            (d0 + 1, 512, 384, 128, True),
            (d0 + 3, 896, 128, 384, True),
            (d0 + 2, 1024, 256, 256, True),
        ]
        batches.append((1280, diag))
        nd = list(range(0, kpc * c))
    else:
        nd = list(range(0, S // KT))
    for g in range(0, len(nd), 3):
        grp = nd[g : g + 3]
        batches.append(
            (512 * len(grp), [(j, i * 512, 512, 0, False) for i, j in enumerate(grp)])
        )
    return batches


def _build(causal):
    nc = bacc.Bacc(None, target_bir_lowering=False)
    # All DRAM I/O is f32-typed (bf16 host arrays hang the axon transport);
    # qt/kt/va carry bf16 PAIRS packed into f32 words, unpacked on device
    # for free via AP.bitcast views.  Big contiguous descriptors only.
    #
    # PE packing: QK matmuls contract over d=64, so Q^T/K^T arrive
    # duplicated on partitions 64..127 and consecutive QK matmuls run as
    # concurrent row-group tenants (measured ~108 ns per N=512 matmul vs
    # 379 ns unpacked).  PV splits each k-tile's contraction into rows
    # 0-63 / 64-127 accumulating into two psum banks (accA/accB) that one
    # DVE add merges per chunk — this hides LDWEIGHTS the same way and
    # keeps the ones-column denominator trick.
    njt = S // KT  # k-tiles per head
    VW = D + 1  # V columns incl. the baked-in ones column
    qt = nc.declare_dram_parameter("qt", [HPC, 2 * D, S // 2], F32, isOutput=False)
    kt = nc.declare_dram_parameter("kt", [HPC, 2 * D, S // 2], F32, isOutput=False)
    va = nc.declare_dram_parameter("va", [HPC, KT, njt * VW // 2], F32, isOutput=False)
    o = nc.declare_dram_parameter("o", [HPC, VW, S], F32, isOutput=True)

    nchunks = S // CH

    with tile.TileContext(nc) as tc:
        with (
            tc.tile_pool(name="const", bufs=1) as const,
            tc.tile_pool(name="qk", bufs=2) as qk_pool,
            tc.tile_pool(name="vaug", bufs=2) as vaug_pool,
            tc.tile_pool(name="pt", bufs=3) as pt_pool,
            tc.tile_pool(name="osb", bufs=2) as osb_pool,
            tc.tile_pool(name="st", bufs=2, space="PSUM") as st_pool,
            tc.tile_pool(name="acc", bufs=1, space="PSUM") as acc_pool,
        ):
            mask = const.tile([KT, CH], F32)
            nc.gpsimd.memset(mask, 0.0)
            # keep 0 where free - part >= 0 (q >= k), else NEG
            nc.gpsimd.affine_select(
                out=mask,
                in_=mask,
                compare_op=mybir.AluOpType.is_ge,
                fill=NEG,
                base=0,
                pattern=[[1, CH]],
                channel_multiplier=-1,
            )

            for h in range(HPC):
                qt_sb = qk_pool.tile([2 * D, S], BF16, tag="qt")
                kt_sb = qk_pool.tile([2 * D, S], BF16, tag="kt")
                nc.sync.dma_start(out=qt_sb.bitcast(F32), in_=qt[h])
                nc.sync.dma_start(out=kt_sb.bitcast(F32), in_=kt[h])
                v_aug = vaug_pool.tile([KT, njt * VW], BF16)
                nc.sync.dma_start(out=v_aug.bitcast(F32), in_=va[h])

                o_sb = osb_pool.tile([VW, S], F32)
                for c in range(nchunks):
                    accA = acc_pool.tile([VW, CH], F32, tag="accA")
                    accB = acc_pool.tile([VW, CH], F32, tag="accB")
                    batches = _plan_chunk(c, causal)
                    n_pv = sum(len(b[1]) for b in batches)
                    pv_i = 0
                    qk_parity = 0
                    for bw, blocks in batches:
                        st = st_pool.tile([KT, 1536], F32, tag="st")
                        for j, off, span, qlo, diag in blocks:
                            p0 = D * qk_parity  # row-group tenant 0 or 64
                            qk_parity ^= 1
                            nc.tensor.matmul(
                                st[:, off : off + span],
                                lhsT=kt_sb[p0 : p0 + D, j * KT : (j + 1) * KT],
                                rhs=qt_sb[
                                    p0 : p0 + D,
                                    c * CH + qlo : c * CH + qlo + span,
                                ],
                                start=True,
                                stop=True,
                            )
                            if diag:
                                nc.vector.tensor_add(
                                    st[:, off : off + span],
                                    st[:, off : off + span],
                                    mask[:, :span],
                                )
                        pt = pt_pool.tile([KT, 1536], BF16, tag="pt")
                        nc.scalar.activation(
                            pt[:, :bw],
                            st[:, :bw],
                            mybir.ActivationFunctionType.Exp,
                            scale=float(1.0 / np.sqrt(D)),
                        )
                        for j, off, span, qlo, diag in blocks:
                            jc = j * VW
                            nc.tensor.matmul(
                                accA[:, qlo : qlo + span],
                                lhsT=v_aug[0:D, jc : jc + VW],
                                rhs=pt[0:D, off : off + span],
                                start=(pv_i == 0),
                                stop=(pv_i == n_pv - 1),
                            )
                            nc.tensor.matmul(
                                accB[:, qlo : qlo + span],
                                lhsT=v_aug[D : 2 * D, jc : jc + VW],
                                rhs=pt[D : 2 * D, off : off + span],
                                start=(pv_i == 0),
                                stop=(pv_i == n_pv - 1),
                            )
                            pv_i += 1
                    # DVE may read only one PSUM operand per instruction
                    mrg = osb_pool.tile([VW, CH], F32, tag="mrg")
                    nc.vector.tensor_copy(mrg, accB)
                    nc.vector.tensor_add(
                        o_sb[:, c * CH : (c + 1) * CH], accA, mrg
                    )
                nc.sync.dma_start(out=o[h], in_=o_sb)
    nc.compile()
    return nc


_CACHE = {}


def _get_nc(causal):
    if causal not in _CACHE:
        _CACHE[causal] = _build(causal)
    return _CACHE[causal]


def _prep_inputs(q, k, v):
    """Shard + pre-transpose + bf16-pack on host -> per-core in_maps.

    qt/kt: head-major [BH, D, S] bf16, adjacent pairs packed into f32.
    va: v_aug [BH, 128, njt*65] bf16 (v tiles k-major on partitions with a
    ones column per tile), packed into f32 the same way.
    """
    import ml_dtypes

    njt = S // KT
    VW = D + 1
    q = np.asarray(q, dtype=np.float32).reshape(B * H, S, D)
    k = np.asarray(k, dtype=np.float32).reshape(B * H, S, D)
    v = np.asarray(v, dtype=np.float32).reshape(B * H, S, D)
    qt1 = np.ascontiguousarray(q.transpose(0, 2, 1)).astype(ml_dtypes.bfloat16)
    kt1 = np.ascontiguousarray(k.transpose(0, 2, 1)).astype(ml_dtypes.bfloat16)
    # duplicate on partitions 64..127 for the second row-group tenant
    qt = np.concatenate([qt1, qt1], axis=1)  # [BH, 2D, S]
    kt = np.concatenate([kt1, kt1], axis=1)
    va = np.empty((B * H, KT, njt, VW), dtype=ml_dtypes.bfloat16)
    va[..., :D] = v.reshape(B * H, njt, KT, D).transpose(0, 2, 1, 3)
    va[..., D] = 1.0
    qt_p = qt.view(np.float32)  # [BH, 2D, S//2]
    kt_p = kt.view(np.float32)
    va_p = va.reshape(B * H, KT, njt * VW).view(np.float32)
    in_maps = []
    for i in range(N_CORES):
        sl = slice(i * HPC, (i + 1) * HPC)
        in_maps.append(
            {
                "qt": np.ascontiguousarray(qt_p[sl]),
                "kt": np.ascontiguousarray(kt_p[sl]),
                "va": np.ascontiguousarray(va_p[sl]),
            }
        )
    return in_maps


def _postprocess(results):
    """Per-core [HPC, D+1, S] -> full [B, H, S, D] (divide + transpose)."""
    outs = []
    for i in range(N_CORES):
        oc = results[i]["o"]  # [HPC, D+1, S]
        num = oc[:, :D, :]  # [HPC, D, S]
        den = oc[:, D : D + 1, :]  # [HPC, 1, S]
        outs.append((num / den).transpose(0, 2, 1))  # [HPC, S, D]
    return np.concatenate(outs, axis=0).reshape(B, H, S, D).astype(np.float32)


def _run(q, k, v, mask, trace=False):
    mask = np.asarray(mask)
    causal = bool(np.array_equal(mask, np.tril(np.ones((S, S), dtype=bool))))
    if not causal:
        assert mask.all(), (
            "only causal (tril) or all-ones masks are supported by this kernel"
        )
    nc = _get_nc(causal)
    in_maps = _prep_inputs(q, k, v)
    res = run_bass_kernel_spmd(nc, in_maps, list(range(N_CORES)), trace=trace)
    out = _postprocess(res.results)
    return out, res


def kernel(q, k, v, mask):
    out, _ = _run(q, k, v, mask, trace=False)
    return out



# revision 4
# speedup vs baseline: 1.2887x; 1.2887x over previous
"""Causal multi-head attention on 8 Trainium2 NeuronCores (Bass/Tile).

Problem: B=4 H=16 S=2048 D=64 fp32, causal mask, softmax(QK^T/sqrt(D))V.
Sharding: batch*heads (64) split 8 per core; no cross-core communication.

Design notes (v2)
-----------------
- Host pre-transposes Q,K to [d, s] per head so the device needs zero
  transposes; scores are computed TRANSPOSED (S^T[k, q]) so softmax's
  P^T is directly the moving operand of the P@V matmul.
- Softmax over k (= partition dim in S^T) avoids max-subtraction (scores
  ~N(0,1) after 1/sqrt(64) scaling) and gets the denominator free via a
  ones-column appended to V.  Final divide + transpose happen on host.
- QK matmuls contract over d=64 and run as two concurrent row-group
  tenants (Q/K duplicated on partitions 64..127) -> ~2 cols/cycle.
- PV runs single-tenant K=128 into ONE psum bank per chunk (acc pool
  bufs=2 double-buffers across chunks); the old dual-tenant accA/accB +
  DVE merge is gone - one DVE copy psum->sbuf per chunk remains.
- Causal masking: only the [128,128] diagonal square of each diagonal
  block differs from zero, so the DVE additive mask covers 128 cols per
  diag tile instead of the full span (2.6x less DVE work), off the
  scalar engine's critical path.
- Emission is software-pipelined: each batch's PV is emitted AFTER the
  next batch's QK+exp, so the scalar engine (the throughput floor at
  ~1 elem/lane/cycle for exp) stays saturated and the PE never waits
  on an ACTIVATE it just enqueued.
- All matmuls bf16 (fp32 PE matmuls stream multi-pass, ~3x slower);
  fp32 accumulation in PSUM; exp computed in fp32 from PSUM.
"""

import os
import sys

import numpy as np

sys.path.insert(0, "/opt/trn_rl_repo")

import concourse.bass as bass  # noqa: E402
import concourse.tile as tile  # noqa: E402
from concourse import bacc, mybir  # noqa: E402
from concourse.bass_utils import run_bass_kernel_spmd  # noqa: E402

B, H, S, D = 4, 16, 2048, 64
N_CORES = 8
HPC = (B * H) // N_CORES  # heads per core
KT = 128   # k-tile rows
CH = 512   # q-chunk cols
NEG = -1e9

F32 = mybir.dt.float32
BF16 = mybir.dt.bfloat16


def _plan_chunk(c, causal):
    """Per q-chunk list of ACTIVATE batches.

    Each batch is (width, [(j, off, span, qlo, diag), ...]): k-tile j's
    scores for q-columns [qlo, qlo+span) of the chunk land at packed psum
    columns [off, off+span).  Offsets never let a matmul cross a 512-col
    psum bank boundary.  `diag` marks blocks needing the causal mask.
    Non-diagonal batches come first so each chunk's pipeline starts with
    mask-free work; the diagonal batch (with its DVE mask adds) is last.
    """
    kpc = CH // KT  # k-tiles per chunk (4)
    batches = []
    if causal:
        nd = list(range(0, kpc * c))
    else:
        nd = list(range(0, S // KT))
    for g in range(0, len(nd), 3):
        grp = nd[g : g + 3]
        batches.append(
            (512 * len(grp), [(j, i * 512, 512, 0, False) for i, j in enumerate(grp)])
        )
    if causal:
        # diagonal k-tiles j=kpc*c+r; packed order r0,r1,r3,r2 fills
        # [0,1280) with every matmul within a psum bank
        d0 = kpc * c
        diag = [
            (d0 + 0, 0, 512, 0, True),
            (d0 + 1, 512, 384, 128, True),
            (d0 + 3, 896, 128, 384, True),
            (d0 + 2, 1024, 256, 256, True),
        ]
        batches.append((1280, diag))
    return batches


def _build(causal):
    nc = bacc.Bacc(None, target_bir_lowering=False)
    # All DRAM I/O is f32-typed (bf16 host arrays hang the axon transport);
    # qt/kt/va carry bf16 PAIRS packed into f32 words, unpacked on device
    # for free via AP.bitcast views.  Big contiguous descriptors only.
    njt = S // KT  # k-tiles per head
    VW = D + 1  # V columns incl. the baked-in ones column
    qt = nc.declare_dram_parameter("qt", [HPC, 2 * D, S // 2], F32, isOutput=False)
    kt = nc.declare_dram_parameter("kt", [HPC, 2 * D, S // 2], F32, isOutput=False)
    va = nc.declare_dram_parameter("va", [HPC, KT, njt * VW // 2], F32, isOutput=False)
    o = nc.declare_dram_parameter("o", [HPC, VW, S], F32, isOutput=True)

    nchunks = S // CH

    with tile.TileContext(nc) as tc:
        with (
            tc.tile_pool(name="const", bufs=1) as const,
            tc.tile_pool(name="qk", bufs=2) as qk_pool,
            tc.tile_pool(name="vaug", bufs=2) as vaug_pool,
            tc.tile_pool(name="pt", bufs=3) as pt_pool,
            tc.tile_pool(name="osb", bufs=2) as osb_pool,
            tc.tile_pool(name="st", bufs=2, space="PSUM") as st_pool,
            tc.tile_pool(name="acc", bufs=2, space="PSUM") as acc_pool,
        ):
            mask = const.tile([KT, KT], F32)
            nc.gpsimd.memset(mask, 0.0)
            # keep 0 where free - part >= 0 (q >= k), else NEG
            nc.gpsimd.affine_select(
                out=mask,
                in_=mask,
                compare_op=mybir.AluOpType.is_ge,
                fill=NEG,
                base=0,
                pattern=[[1, KT]],
                channel_multiplier=-1,
            )

            for h in range(HPC):
                qt_sb = qk_pool.tile([2 * D, S], BF16, tag="qt")
                kt_sb = qk_pool.tile([2 * D, S], BF16, tag="kt")
                nc.sync.dma_start(out=qt_sb.bitcast(F32), in_=qt[h])
                nc.sync.dma_start(out=kt_sb.bitcast(F32), in_=kt[h])
                v_aug = vaug_pool.tile([KT, njt * VW], BF16)
                nc.sync.dma_start(out=v_aug.bitcast(F32), in_=va[h])

                o_sb = osb_pool.tile([VW, S], F32)

                # Flatten all (chunk, batch) work items for this head.
                work = []  # (c, acc_first, acc_last, bw, blocks)
                for c in range(nchunks):
                    batches = _plan_chunk(c, causal)
                    for bi, (bw, blocks) in enumerate(batches):
                        work.append(
                            (c, bi == 0, bi == len(batches) - 1, bw, blocks)
                        )

                accs = {}  # chunk -> acc tile

                def emit_pv(item):
                    c, first, last, bw, blocks, pt = item
                    acc = accs[c]
                    n = len(blocks)
                    for i, (j, off, span, qlo, diag) in enumerate(blocks):
                        jc = j * VW
                        nc.tensor.matmul(
                            acc[:, qlo : qlo + span],
                            lhsT=v_aug[0:KT, jc : jc + VW],
                            rhs=pt[0:KT, off : off + span],
                            start=(first and i == 0),
                            stop=(last and i == n - 1),
                        )
                    if last:
                        nc.vector.tensor_copy(
                            o_sb[:, c * CH : (c + 1) * CH], acc
                        )

                pending = None
                qk_parity = 0
                for item in work:
                    c, first, last, bw, blocks = item
                    if first:
                        accs[c] = acc_pool.tile(
                            [VW, CH], F32, tag="acc", name="acc"
                        )
                    st = st_pool.tile([KT, 1536], F32, tag="st")
                    for j, off, span, qlo, diag in blocks:
                        p0 = D * qk_parity  # row-group tenant 0 or 64
                        qk_parity ^= 1
                        nc.tensor.matmul(
                            st[:, off : off + span],
                            lhsT=kt_sb[p0 : p0 + D, j * KT : (j + 1) * KT],
                            rhs=qt_sb[
                                p0 : p0 + D,
                                c * CH + qlo : c * CH + qlo + span,
                            ],
                            start=True,
                            stop=True,
                        )
                        if diag:
                            # only the first 128 cols of a diagonal block
                            # intersect the causal boundary
                            nc.vector.tensor_add(
                                st[:, off : off + KT],
                                st[:, off : off + KT],
                                mask,
                            )
                    pt = pt_pool.tile([KT, 1536], BF16, tag="pt")
                    nc.scalar.activation(
                        pt[:, :bw],
                        st[:, :bw],
                        mybir.ActivationFunctionType.Exp,
                        scale=float(1.0 / np.sqrt(D)),
                    )
                    if pending is not None:
                        emit_pv(pending)
                    pending = item + (pt,)
                if pending is not None:
                    emit_pv(pending)
                nc.sync.dma_start(out=o[h], in_=o_sb)
    nc.compile()
    return nc


_CACHE = {}


def _get_nc(causal):
    if causal not in _CACHE:
        _CACHE[causal] = _build(causal)
    return _CACHE[causal]


def _prep_inputs(q, k, v):
    """Shard + pre-transpose + bf16-pack on host -> per-core in_maps.

    qt/kt: head-major [BH, D, S] bf16, adjacent pairs packed into f32.
    va: v_aug [BH, 128, njt*65] bf16 (v tiles k-major on partitions with a
    ones column per tile), packed into f32 the same way.
    """
    import ml_dtypes

    njt = S // KT
    VW = D + 1
    q = np.asarray(q, dtype=np.float32).reshape(B * H, S, D)
    k = np.asarray(k, dtype=np.float32).reshape(B * H, S, D)
    v = np.asarray(v, dtype=np.float32).reshape(B * H, S, D)
    qt1 = np.ascontiguousarray(q.transpose(0, 2, 1)).astype(ml_dtypes.bfloat16)
    kt1 = np.ascontiguousarray(k.transpose(0, 2, 1)).astype(ml_dtypes.bfloat16)
    # duplicate on partitions 64..127 for the second row-group tenant
    qt = np.concatenate([qt1, qt1], axis=1)  # [BH, 2D, S]
    kt = np.concatenate([kt1, kt1], axis=1)
    va = np.empty((B * H, KT, njt, VW), dtype=ml_dtypes.bfloat16)
    va[..., :D] = v.reshape(B * H, njt, KT, D).transpose(0, 2, 1, 3)
    va[..., D] = 1.0
    qt_p = qt.view(np.float32)  # [BH, 2D, S//2]
    kt_p = kt.view(np.float32)
    va_p = va.reshape(B * H, KT, njt * VW).view(np.float32)
    in_maps = []
    for i in range(N_CORES):
        sl = slice(i * HPC, (i + 1) * HPC)
        in_maps.append(
            {
                "qt": np.ascontiguousarray(qt_p[sl]),
                "kt": np.ascontiguousarray(kt_p[sl]),
                "va": np.ascontiguousarray(va_p[sl]),
            }
        )
    return in_maps


def _postprocess(results):
    """Per-core [HPC, D+1, S] -> full [B, H, S, D] (divide + transpose)."""
    outs = []
    for i in range(N_CORES):
        oc = results[i]["o"]  # [HPC, D+1, S]
        num = oc[:, :D, :]  # [HPC, D, S]
        den = oc[:, D : D + 1, :]  # [HPC, 1, S]
        outs.append((num / den).transpose(0, 2, 1))  # [HPC, S, D]
    return np.concatenate(outs, axis=0).reshape(B, H, S, D).astype(np.float32)


def _run(q, k, v, mask, trace=False):
    mask = np.asarray(mask)
    causal = bool(np.array_equal(mask, np.tril(np.ones((S, S), dtype=bool))))
    if not causal:
        assert mask.all(), (
            "only causal (tril) or all-ones masks are supported by this kernel"
        )
    nc = _get_nc(causal)
    in_maps = _prep_inputs(q, k, v)
    res = run_bass_kernel_spmd(nc, in_maps, list(range(N_CORES)), trace=trace)
    out = _postprocess(res.results)
    return out, res


def kernel(q, k, v, mask):
    out, _ = _run(q, k, v, mask, trace=False)
    return out


# revision 12
# speedup vs baseline: 1.6088x; 1.2484x over previous
"""Causal multi-head attention on 8 Trainium2 NeuronCores (Bass/Tile).

Problem: B=4 H=16 S=2048 D=64 fp32, causal mask, softmax(QK^T/sqrt(D))V.
Sharding: batch*heads (64) split 8 per core; no cross-core communication.

Design notes (v2)
-----------------
- Host pre-transposes Q,K to [d, s] per head so the device needs zero
  transposes; scores are computed TRANSPOSED (S^T[k, q]) so softmax's
  P^T is directly the moving operand of the P@V matmul.
- Softmax over k (= partition dim in S^T) avoids max-subtraction (scores
  ~N(0,1) after 1/sqrt(64) scaling) and gets the denominator free via a
  ones-column appended to V.  Final divide + transpose happen on host.
- QK matmuls contract over d=64 and run as two concurrent row-group
  tenants (Q/K duplicated on partitions 64..127) -> ~2 cols/cycle.
- PV runs single-tenant K=128 into ONE psum bank per chunk (acc pool
  bufs=2 double-buffers across chunks); the old dual-tenant accA/accB +
  DVE merge is gone - one DVE copy psum->sbuf per chunk remains.
- Causal masking: only the [128,128] diagonal square of each diagonal
  block differs from zero, so the DVE additive mask covers 128 cols per
  diag tile instead of the full span (2.6x less DVE work), off the
  scalar engine's critical path.
- Emission is software-pipelined: each batch's PV is emitted AFTER the
  next batch's QK+exp, so the scalar engine (the throughput floor at
  ~1 elem/lane/cycle for exp) stays saturated and the PE never waits
  on an ACTIVATE it just enqueued.
- All matmuls bf16 (fp32 PE matmuls stream multi-pass, ~3x slower);
  fp32 accumulation in PSUM; exp computed in fp32 from PSUM.
"""

import os
import sys

import numpy as np

sys.path.insert(0, "/opt/trn_rl_repo")

import concourse.bass as bass  # noqa: E402
import concourse.tile as tile  # noqa: E402
from concourse import bacc, mybir  # noqa: E402
from concourse.bass_utils import run_bass_kernel_spmd  # noqa: E402

B, H, S, D = 4, 16, 2048, 64
N_CORES = 8
HPC = (B * H) // N_CORES  # heads per core
KT = 128   # k-tile rows
CH = 512   # q-chunk cols
NEG = -1e9

F32 = mybir.dt.float32
BF16 = mybir.dt.bfloat16


def _plan_chunk(c, causal):
    """Per q-chunk list of ACTIVATE batches.

    Each batch is (width, [(j, off, span, qlo, diag), ...]): k-tile j's
    scores for q-columns [qlo, qlo+span) of the chunk land at packed psum
    columns [off, off+span).  Offsets never let a matmul cross a 512-col
    psum bank boundary.  `diag` marks blocks needing the causal mask.
    Non-diagonal batches come first so each chunk's pipeline starts with
    mask-free work; the diagonal batch (with its DVE mask adds) is last.
    """
    kpc = CH // KT  # k-tiles per chunk (4)
    batches = []
    if causal:
        nd = list(range(0, kpc * c))
    else:
        nd = list(range(0, S // KT))
    # split into groups of <=3 (psum budget), preferring even group sizes so
    # dual-tenant QK pairs never run unpaired
    if len(nd) % 3 == 1 and len(nd) >= 4:
        sizes = [3] * (len(nd) // 3 - 1) + [2, 2]
    else:
        sizes = [3] * (len(nd) // 3) + ([len(nd) % 3] if len(nd) % 3 else [])
    g = 0
    for sz in sizes:
        grp = nd[g : g + sz]
        g += sz
        batches.append(
            (512 * len(grp), [(j, i * 512, 512, 0, False) for i, j in enumerate(grp)])
        )
    if causal:
        # diagonal k-tiles j=kpc*c+r; packed order r0,r1,r3,r2 fills
        # [0,1280) with every matmul within a psum bank
        d0 = kpc * c
        diag = [
            (d0 + 0, 0, 512, 0, True),
            (d0 + 1, 512, 384, 128, True),
            (d0 + 3, 896, 128, 384, True),
            (d0 + 2, 1024, 256, 256, True),
        ]
        batches.append((1280, diag))
    return batches


def _build(causal):
    nc = bacc.Bacc(None, target_bir_lowering=False)
    # All DRAM I/O is f32-typed (bf16 host arrays hang the axon transport);
    # qt/kt/va carry bf16 PAIRS packed into f32 words, unpacked on device
    # for free via AP.bitcast views.  Big contiguous descriptors only.
    njt = S // KT  # k-tiles per head
    VW = D + 1  # V columns incl. the baked-in ones column
    qt = nc.declare_dram_parameter("qt", [HPC, 2 * D, S // 2], F32, isOutput=False)
    kt = nc.declare_dram_parameter("kt", [HPC, 2 * D, S // 2], F32, isOutput=False)
    va = nc.declare_dram_parameter("va", [HPC, KT, njt * VW // 2], F32, isOutput=False)
    # cm: [128, 128+512] bf16 packed in f32 pairs - identity (cols 0:128)
    # then the additive causal mask (cols 128:640, NEG strictly above diag)
    cm = nc.declare_dram_parameter("cm", [KT, (KT + CH) // 2], F32, isOutput=False)
    o = nc.declare_dram_parameter("o", [HPC, VW, S], F32, isOutput=True)

    nchunks = S // CH

    with tile.TileContext(nc) as tc:
        with (
            tc.tile_pool(name="const", bufs=1) as const,
            tc.tile_pool(name="qk", bufs=2) as qk_pool,
            tc.tile_pool(name="vaug", bufs=2) as vaug_pool,
            tc.tile_pool(name="pt", bufs=4) as pt_pool,
            tc.tile_pool(name="osb", bufs=2) as osb_pool,
            tc.tile_pool(name="st", bufs=2, space="PSUM") as st_pool,
            tc.tile_pool(name="acc", bufs=2, space="PSUM") as acc_pool,
        ):
            cm_sb = const.tile([KT, KT + CH], BF16)
            nc.sync.dma_start(out=cm_sb.bitcast(F32), in_=cm[0:KT])
            ident = cm_sb[:, 0:KT]
            negmask = cm_sb[:, KT : KT + CH]

            # Input DMAs are issued one head ahead so the (program-order
            # earlier) output DMA of head h never blocks head h+1's loads
            # on the sync queue.
            def load_head(h):
                qt_sb = qk_pool.tile([2 * D, S], BF16, tag="qt", name="qt_sb")
                kt_sb = qk_pool.tile([2 * D, S], BF16, tag="kt", name="kt_sb")
                nc.sync.dma_start(out=qt_sb.bitcast(F32), in_=qt[h])
                nc.sync.dma_start(out=kt_sb.bitcast(F32), in_=kt[h])
                v_aug = vaug_pool.tile(
                    [KT, njt * VW], BF16, tag="va", name="v_aug"
                )
                nc.sync.dma_start(out=v_aug.bitcast(F32), in_=va[h])
                return qt_sb, kt_sb, v_aug

            nxt = load_head(0)
            for h in range(HPC):
                qt_sb, kt_sb, v_aug = nxt
                if h + 1 < HPC:
                    nxt = load_head(h + 1)

                o_sb = osb_pool.tile([VW, S], F32)

                # Flatten all (chunk, batch) work items for this head.
                work = []  # (c, acc_first, acc_last, bw, blocks)
                for c in range(nchunks):
                    batches = _plan_chunk(c, causal)
                    for bi, (bw, blocks) in enumerate(batches):
                        work.append(
                            (c, bi == 0, bi == len(batches) - 1, bw, blocks)
                        )

                accs = {}  # chunk -> acc tile

                def emit_pv(item):
                    c, first, last, bw, blocks, pt = item
                    acc = accs[c]
                    n = len(blocks)
                    for i, (j, off, span, qlo, diag) in enumerate(blocks):
                        jc = j * VW
                        nc.tensor.matmul(
                            acc[:, qlo : qlo + span],
                            lhsT=v_aug[0:KT, jc : jc + VW],
                            rhs=pt[0:KT, off : off + span],
                            start=(first and i == 0),
                            stop=(last and i == n - 1),
                        )
                    if last:
                        nc.vector.tensor_copy(
                            o_sb[:, c * CH : (c + 1) * CH], acc
                        )

                pending = None
                qk_parity = 0
                for item in work:
                    c, first, last, bw, blocks = item
                    if first:
                        accs[c] = acc_pool.tile(
                            [VW, CH], F32, tag="acc", name="acc"
                        )
                    st = st_pool.tile([KT, 1536], F32, tag="st")
                    for j, off, span, qlo, diag in blocks:
                        p0 = D * qk_parity  # row-group tenant 0 or 64
                        qk_parity ^= 1
                        nc.tensor.matmul(
                            st[:, off : off + span],
                            lhsT=kt_sb[p0 : p0 + D, j * KT : (j + 1) * KT],
                            rhs=qt_sb[
                                p0 : p0 + D,
                                c * CH + qlo : c * CH + qlo + span,
                            ],
                            start=True,
                            stop=not diag,
                        )
                        if diag:
                            # causal mask applied by the PE itself:
                            # st += I.T @ negmask.  Emitted directly after
                            # its QK matmul - a later start=True to the same
                            # psum bank clears has_written bank-wide, which
                            # would turn this accumulate into an overwrite.
                            nc.tensor.matmul(
                                st[:, off : off + span],
                                lhsT=ident,
                                rhs=negmask[:, 0:span],
                                start=False,
                                stop=True,
                            )
                    pt = pt_pool.tile([KT, 1536], BF16, tag="pt")
                    nc.scalar.activation(
                        pt[:, :bw],
                        st[:, :bw],
                        mybir.ActivationFunctionType.Exp,
                        scale=float(1.0 / np.sqrt(D)),
                    )
                    if pending is not None:
                        emit_pv(pending)
                    pending = item + (pt,)
                if pending is not None:
                    emit_pv(pending)
                nc.sync.dma_start(out=o[h], in_=o_sb)
    nc.compile()
    return nc


_CACHE = {}


def _get_nc(causal):
    if causal not in _CACHE:
        _CACHE[causal] = _build(causal)
    return _CACHE[causal]


def _prep_inputs(q, k, v):
    """Shard + pre-transpose + bf16-pack on host -> per-core in_maps.

    qt/kt: head-major [BH, D, S] bf16, adjacent pairs packed into f32.
    va: v_aug [BH, 128, njt*65] bf16 (v tiles k-major on partitions with a
    ones column per tile), packed into f32 the same way.
    """
    import ml_dtypes

    njt = S // KT
    VW = D + 1
    q = np.asarray(q, dtype=np.float32).reshape(B * H, S, D)
    k = np.asarray(k, dtype=np.float32).reshape(B * H, S, D)
    v = np.asarray(v, dtype=np.float32).reshape(B * H, S, D)
    qt1 = np.ascontiguousarray(q.transpose(0, 2, 1)).astype(ml_dtypes.bfloat16)
    kt1 = np.ascontiguousarray(k.transpose(0, 2, 1)).astype(ml_dtypes.bfloat16)
    # duplicate on partitions 64..127 for the second row-group tenant
    qt = np.concatenate([qt1, qt1], axis=1)  # [BH, 2D, S]
    kt = np.concatenate([kt1, kt1], axis=1)
    va = np.empty((B * H, KT, njt, VW), dtype=ml_dtypes.bfloat16)
    va[..., :D] = v.reshape(B * H, njt, KT, D).transpose(0, 2, 1, 3)
    va[..., D] = 1.0
    qt_p = qt.view(np.float32)  # [BH, 2D, S//2]
    kt_p = kt.view(np.float32)
    va_p = va.reshape(B * H, KT, njt * VW).view(np.float32)
    # identity + additive causal mask, streamed through the PE on device
    cmh = np.zeros((KT, KT + CH), dtype=ml_dtypes.bfloat16)
    cmh[:, :KT] = np.eye(KT, dtype=np.float32)
    i_idx = np.arange(KT)[:, None]
    j_idx = np.arange(CH)[None, :]
    cmh[:, KT:] = np.where(j_idx >= i_idx, 0.0, NEG).astype(ml_dtypes.bfloat16)
    cm_p = np.ascontiguousarray(cmh.view(np.float32))
    in_maps = []
    for i in range(N_CORES):
        sl = slice(i * HPC, (i + 1) * HPC)
        in_maps.append(
            {
                "qt": np.ascontiguousarray(qt_p[sl]),
                "kt": np.ascontiguousarray(kt_p[sl]),
                "va": np.ascontiguousarray(va_p[sl]),
                "cm": cm_p,
            }
        )
    return in_maps


def _postprocess(results):
    """Per-core [HPC, D+1, S] -> full [B, H, S, D] (divide + transpose)."""
    outs = []
    for i in range(N_CORES):
        oc = results[i]["o"]  # [HPC, D+1, S]
        num = oc[:, :D, :]  # [HPC, D, S]
        den = oc[:, D : D + 1, :]  # [HPC, 1, S]
        outs.append((num / den).transpose(0, 2, 1))  # [HPC, S, D]
    return np.concatenate(outs, axis=0).reshape(B, H, S, D).astype(np.float32)


def _run(q, k, v, mask, trace=False):
    mask = np.asarray(mask)
    causal = bool(np.array_equal(mask, np.tril(np.ones((S, S), dtype=bool))))
    if not causal:
        assert mask.all(), (
            "only causal (tril) or all-ones masks are supported by this kernel"
        )
    nc = _get_nc(causal)
    in_maps = _prep_inputs(q, k, v)
    res = run_bass_kernel_spmd(nc, in_maps, list(range(N_CORES)), trace=trace)
    out = _postprocess(res.results)
    return out, res


def kernel(q, k, v, mask):
    out, _ = _run(q, k, v, mask, trace=False)
    return out


# revision 13
# speedup vs baseline: 2.1995x; 1.3671x over previous
"""Causal multi-head attention on 8 Trainium2 NeuronCores (Bass/Tile).

Problem: B=4 H=16 S=2048 D=64 fp32, causal mask, softmax(QK^T/sqrt(D))V.
Sharding: batch*heads (64) split 8 per core; no cross-core communication.

Design notes (v2)
-----------------
- Host pre-transposes Q,K to [d, s] per head so the device needs zero
  transposes; scores are computed TRANSPOSED (S^T[k, q]) so softmax's
  P^T is directly the moving operand of the P@V matmul.
- Softmax over k (= partition dim in S^T) avoids max-subtraction (scores
  ~N(0,1) after 1/sqrt(64) scaling) and gets the denominator free via a
  ones-column appended to V.  Final divide + transpose happen on host.
- QK matmuls contract over d=64 and run as two concurrent row-group
  tenants (Q/K duplicated on partitions 64..127) -> ~2 cols/cycle.
- PV runs single-tenant K=128 into ONE psum bank per chunk (acc pool
  bufs=2 double-buffers across chunks); the old dual-tenant accA/accB +
  DVE merge is gone - one DVE copy psum->sbuf per chunk remains.
- Causal masking: only the [128,128] diagonal square of each diagonal
  block differs from zero, so the DVE additive mask covers 128 cols per
  diag tile instead of the full span (2.6x less DVE work), off the
  scalar engine's critical path.
- Emission is software-pipelined: each batch's PV is emitted AFTER the
  next batch's QK+exp, so the scalar engine (the throughput floor at
  ~1 elem/lane/cycle for exp) stays saturated and the PE never waits
  on an ACTIVATE it just enqueued.
- All matmuls bf16 (fp32 PE matmuls stream multi-pass, ~3x slower);
  fp32 accumulation in PSUM; exp computed in fp32 from PSUM.
"""

import os
import sys

import numpy as np

sys.path.insert(0, "/opt/trn_rl_repo")

import concourse.bass as bass  # noqa: E402
import concourse.tile as tile  # noqa: E402
from concourse import bacc, mybir  # noqa: E402
from concourse.bass_utils import run_bass_kernel_spmd  # noqa: E402

B, H, S, D = 4, 16, 2048, 64
N_CORES = 8
HPC = (B * H) // N_CORES  # heads per core
KT = 128   # k-tile rows
CH = 512   # q-chunk cols
NEG = -1e9

F32 = mybir.dt.float32
BF16 = mybir.dt.bfloat16


def _plan_chunk(c, causal):
    """Per q-chunk list of ACTIVATE batches.

    Each batch is (width, [(j, off, span, qlo, diag), ...]): k-tile j's
    scores for q-columns [qlo, qlo+span) of the chunk land at packed psum
    columns [off, off+span).  Offsets never let a matmul cross a 512-col
    psum bank boundary.  `diag` marks blocks needing the causal mask.
    Non-diagonal batches come first so each chunk's pipeline starts with
    mask-free work; the diagonal batch (with its DVE mask adds) is last.
    """
    kpc = CH // KT  # k-tiles per chunk (4)
    batches = []
    if causal:
        nd = list(range(0, kpc * c))
    else:
        nd = list(range(0, S // KT))
    # split into groups of <=3 (psum budget), preferring even group sizes so
    # dual-tenant QK pairs never run unpaired
    if len(nd) % 3 == 1 and len(nd) >= 4:
        sizes = [3] * (len(nd) // 3 - 1) + [2, 2]
    else:
        sizes = [3] * (len(nd) // 3) + ([len(nd) % 3] if len(nd) % 3 else [])
    g = 0
    for sz in sizes:
        grp = nd[g : g + sz]
        g += sz
        batches.append(
            (512 * len(grp), [(j, i * 512, 512, 0, False) for i, j in enumerate(grp)])
        )
    if causal:
        # diagonal k-tiles j=kpc*c+r; packed order r0,r1,r3,r2 fills
        # [0,1280) with every matmul within a psum bank
        d0 = kpc * c
        diag = [
            (d0 + 0, 0, 512, 0, True),
            (d0 + 1, 512, 384, 128, True),
            (d0 + 3, 896, 128, 384, True),
            (d0 + 2, 1024, 256, 256, True),
        ]
        batches.append((1280, diag))
    return batches


def _build(causal):
    nc = bacc.Bacc(None, target_bir_lowering=False)
    # All DRAM I/O is f32-typed (bf16 host arrays hang the axon transport);
    # qt/kt/va carry bf16 PAIRS packed into f32 words, unpacked on device
    # for free via AP.bitcast views.  Big contiguous descriptors only.
    njt = S // KT  # k-tiles per head
    VW = D + 1  # V columns incl. the baked-in ones column
    qt = nc.declare_dram_parameter("qt", [HPC, 2 * D, S // 2], F32, isOutput=False)
    kt = nc.declare_dram_parameter("kt", [HPC, 2 * D, S // 2], F32, isOutput=False)
    va = nc.declare_dram_parameter("va", [HPC, KT, njt * VW // 2], F32, isOutput=False)
    # cm: [128, 128+512] bf16 packed in f32 pairs - identity (cols 0:128)
    # then the additive causal mask (cols 128:640, NEG strictly above diag)
    cm = nc.declare_dram_parameter("cm", [KT, (KT + CH) // 2], F32, isOutput=False)
    o = nc.declare_dram_parameter("o", [HPC, VW, S], F32, isOutput=True)

    nchunks = S // CH

    with tile.TileContext(nc) as tc:
        with (
            tc.tile_pool(name="const", bufs=1) as const,
            tc.tile_pool(name="qk", bufs=2) as qk_pool,
            tc.tile_pool(name="vaug", bufs=2) as vaug_pool,
            tc.tile_pool(name="pt", bufs=4) as pt_pool,
            tc.tile_pool(name="osb", bufs=2) as osb_pool,
            tc.tile_pool(name="st", bufs=2, space="PSUM") as st_pool,
            tc.tile_pool(name="acc", bufs=2, space="PSUM") as acc_pool,
        ):
            cm_sb = const.tile([KT, KT + CH], BF16)
            nc.sync.dma_start(out=cm_sb.bitcast(F32), in_=cm[0:KT])
            ident = cm_sb[:, 0:KT]
            negmask = cm_sb[:, KT : KT + CH]

            # Input DMAs are issued one head ahead so the (program-order
            # earlier) output DMA of head h never blocks head h+1's loads
            # on the sync queue.
            def load_head(h):
                qt_sb = qk_pool.tile([2 * D, S], BF16, tag="qt", name="qt_sb")
                kt_sb = qk_pool.tile([2 * D, S], BF16, tag="kt", name="kt_sb")
                nc.sync.dma_start(out=qt_sb.bitcast(F32), in_=qt[h])
                nc.sync.dma_start(out=kt_sb.bitcast(F32), in_=kt[h])
                v_aug = vaug_pool.tile(
                    [KT, njt * VW], BF16, tag="va", name="v_aug"
                )
                nc.sync.dma_start(out=v_aug.bitcast(F32), in_=va[h])
                return qt_sb, kt_sb, v_aug

            nxt = load_head(0)
            for h in range(HPC):
                qt_sb, kt_sb, v_aug = nxt
                if h + 1 < HPC:
                    nxt = load_head(h + 1)

                o_sb = osb_pool.tile([VW, S], F32)

                # Flatten all (chunk, batch) work items for this head.
                work = []  # (c, acc_first, acc_last, bw, blocks)
                for c in range(nchunks):
                    batches = _plan_chunk(c, causal)
                    for bi, (bw, blocks) in enumerate(batches):
                        work.append(
                            (c, bi == 0, bi == len(batches) - 1, bw, blocks)
                        )

                accs = {}  # chunk -> acc tile

                def emit_pv(item):
                    c, first, last, bw, blocks, pt = item
                    acc = accs[c]
                    n = len(blocks)
                    for i, (j, off, span, qlo, diag) in enumerate(blocks):
                        jc = j * VW
                        nc.tensor.matmul(
                            acc[:, qlo : qlo + span],
                            lhsT=v_aug[0:KT, jc : jc + VW],
                            rhs=pt[0:KT, off : off + span],
                            start=(first and i == 0),
                            stop=(last and i == n - 1),
                        )
                    if last:
                        nc.vector.tensor_copy(
                            o_sb[:, c * CH : (c + 1) * CH], acc
                        )

                pending = None
                qk_parity = 0
                for item in work:
                    c, first, last, bw, blocks = item
                    if first:
                        accs[c] = acc_pool.tile(
                            [VW, CH], F32, tag="acc", name="acc"
                        )
                    st = st_pool.tile([KT, 1536], F32, tag="st")
                    # start=True clears has_written for the WHOLE psum bank,
                    # so only the first matmul touching each bank may carry
                    # it: a later start=True would wipe a sibling block's
                    # bits and turn its pending mask-accumulate into an
                    # overwrite.  start=False on a bank-cleared region still
                    # overwrites (per-element: add where bit set, write
                    # where clear), which is exactly what block r3 needs.
                    started_banks = set()
                    for j, off, span, qlo, diag in blocks:
                        p0 = D * qk_parity  # row-group tenant 0 or 64
                        qk_parity ^= 1
                        bank = off // 512
                        nc.tensor.matmul(
                            st[:, off : off + span],
                            lhsT=kt_sb[p0 : p0 + D, j * KT : (j + 1) * KT],
                            rhs=qt_sb[
                                p0 : p0 + D,
                                c * CH + qlo : c * CH + qlo + span,
                            ],
                            start=bank not in started_banks,
                            stop=not diag,
                        )
                        started_banks.add(bank)
                    # causal mask applied by the PE itself: st += I.T @
                    # negmask (keeps the QK->exp chain off the DVE, whose
                    # psum access serializes against matmuls bank-by-bank)
                    for j, off, span, qlo, diag in blocks:
                        if diag:
                            nc.tensor.matmul(
                                st[:, off : off + span],
                                lhsT=ident,
                                rhs=negmask[:, 0:span],
                                start=False,
                                stop=True,
                            )
                    pt = pt_pool.tile([KT, 1536], BF16, tag="pt")
                    nc.scalar.activation(
                        pt[:, :bw],
                        st[:, :bw],
                        mybir.ActivationFunctionType.Exp,
                        scale=float(1.0 / np.sqrt(D)),
                    )
                    if pending is not None:
                        emit_pv(pending)
                    pending = item + (pt,)
                if pending is not None:
                    emit_pv(pending)
                nc.sync.dma_start(out=o[h], in_=o_sb)
    nc.compile()
    return nc


_CACHE = {}


def _get_nc(causal):
    if causal not in _CACHE:
        _CACHE[causal] = _build(causal)
    return _CACHE[causal]


def _prep_inputs(q, k, v):
    """Shard + pre-transpose + bf16-pack on host -> per-core in_maps.

    qt/kt: head-major [BH, D, S] bf16, adjacent pairs packed into f32.
    va: v_aug [BH, 128, njt*65] bf16 (v tiles k-major on partitions with a
    ones column per tile), packed into f32 the same way.
    """
    import ml_dtypes

    njt = S // KT
    VW = D + 1
    q = np.asarray(q, dtype=np.float32).reshape(B * H, S, D)
    k = np.asarray(k, dtype=np.float32).reshape(B * H, S, D)
    v = np.asarray(v, dtype=np.float32).reshape(B * H, S, D)
    qt1 = np.ascontiguousarray(q.transpose(0, 2, 1)).astype(ml_dtypes.bfloat16)
    kt1 = np.ascontiguousarray(k.transpose(0, 2, 1)).astype(ml_dtypes.bfloat16)
    # duplicate on partitions 64..127 for the second row-group tenant
    qt = np.concatenate([qt1, qt1], axis=1)  # [BH, 2D, S]
    kt = np.concatenate([kt1, kt1], axis=1)
    va = np.empty((B * H, KT, njt, VW), dtype=ml_dtypes.bfloat16)
    va[..., :D] = v.reshape(B * H, njt, KT, D).transpose(0, 2, 1, 3)
    va[..., D] = 1.0
    qt_p = qt.view(np.float32)  # [BH, 2D, S//2]
    kt_p = kt.view(np.float32)
    va_p = va.reshape(B * H, KT, njt * VW).view(np.float32)
    # identity + additive causal mask, streamed through the PE on device
    cmh = np.zeros((KT, KT + CH), dtype=ml_dtypes.bfloat16)
    cmh[:, :KT] = np.eye(KT, dtype=np.float32)
    i_idx = np.arange(KT)[:, None]
    j_idx = np.arange(CH)[None, :]
    cmh[:, KT:] = np.where(j_idx >= i_idx, 0.0, NEG).astype(ml_dtypes.bfloat16)
    cm_p = np.ascontiguousarray(cmh.view(np.float32))
    in_maps = []
    for i in range(N_CORES):
        sl = slice(i * HPC, (i + 1) * HPC)
        in_maps.append(
            {
                "qt": np.ascontiguousarray(qt_p[sl]),
                "kt": np.ascontiguousarray(kt_p[sl]),
                "va": np.ascontiguousarray(va_p[sl]),
                "cm": cm_p,
            }
        )
    return in_maps


def _postprocess(results):
    """Per-core [HPC, D+1, S] -> full [B, H, S, D] (divide + transpose)."""
    outs = []
    for i in range(N_CORES):
        oc = results[i]["o"]  # [HPC, D+1, S]
        num = oc[:, :D, :]  # [HPC, D, S]
        den = oc[:, D : D + 1, :]  # [HPC, 1, S]
        outs.append((num / den).transpose(0, 2, 1))  # [HPC, S, D]
    return np.concatenate(outs, axis=0).reshape(B, H, S, D).astype(np.float32)


def _run(q, k, v, mask, trace=False):
    mask = np.asarray(mask)
    causal = bool(np.array_equal(mask, np.tril(np.ones((S, S), dtype=bool))))
    if not causal:
        assert mask.all(), (
            "only causal (tril) or all-ones masks are supported by this kernel"
        )
    nc = _get_nc(causal)
    in_maps = _prep_inputs(q, k, v)
    res = run_bass_kernel_spmd(nc, in_maps, list(range(N_CORES)), trace=trace)
    out = _postprocess(res.results)
    return out, res


def kernel(q, k, v, mask):
    out, _ = _run(q, k, v, mask, trace=False)
    return out


# revision 15
# speedup vs baseline: 2.2762x; 1.0349x over previous
"""Causal multi-head attention on 8 Trainium2 NeuronCores (Bass/Tile).

Problem: B=4 H=16 S=2048 D=64 fp32, causal mask, softmax(QK^T/sqrt(D))V.
Sharding: batch*heads (64) split 8 per core; no cross-core communication.

Design notes (v2)
-----------------
- Host pre-transposes Q,K to [d, s] per head so the device needs zero
  transposes; scores are computed TRANSPOSED (S^T[k, q]) so softmax's
  P^T is directly the moving operand of the P@V matmul.
- Softmax over k (= partition dim in S^T) avoids max-subtraction (scores
  ~N(0,1) after 1/sqrt(64) scaling) and gets the denominator free via a
  ones-column appended to V.  Final divide + transpose happen on host.
- QK matmuls contract over d=64 and run as two concurrent row-group
  tenants (Q/K duplicated on partitions 64..127) -> ~2 cols/cycle.
- PV runs single-tenant K=128 into ONE psum bank per chunk (acc pool
  bufs=2 double-buffers across chunks); the old dual-tenant accA/accB +
  DVE merge is gone - one DVE copy psum->sbuf per chunk remains.
- Causal masking: only the [128,128] diagonal square of each diagonal
  block differs from zero, so the DVE additive mask covers 128 cols per
  diag tile instead of the full span (2.6x less DVE work), off the
  scalar engine's critical path.
- Emission is software-pipelined: each batch's PV is emitted AFTER the
  next batch's QK+exp, so the scalar engine (the throughput floor at
  ~1 elem/lane/cycle for exp) stays saturated and the PE never waits
  on an ACTIVATE it just enqueued.
- All matmuls bf16 (fp32 PE matmuls stream multi-pass, ~3x slower);
  fp32 accumulation in PSUM; exp computed in fp32 from PSUM.
"""

import os
import sys

import numpy as np

sys.path.insert(0, "/opt/trn_rl_repo")

import concourse.bass as bass  # noqa: E402
import concourse.tile as tile  # noqa: E402
from concourse import bacc, mybir  # noqa: E402
from concourse.bass_utils import run_bass_kernel_spmd  # noqa: E402

B, H, S, D = 4, 16, 2048, 64
N_CORES = 8
HPC = (B * H) // N_CORES  # heads per core
KT = 128   # k-tile rows
CH = 512   # q-chunk cols
NEG = -1e9

F32 = mybir.dt.float32
BF16 = mybir.dt.bfloat16


def _plan_chunk(c, causal):
    """Per q-chunk list of ACTIVATE batches.

    Each batch is (width, [(j, off, span, qlo, diag), ...]): k-tile j's
    scores for q-columns [qlo, qlo+span) of the chunk land at packed psum
    columns [off, off+span).  Offsets never let a matmul cross a 512-col
    psum bank boundary.  `diag` marks blocks needing the causal mask.
    Non-diagonal batches come first so each chunk's pipeline starts with
    mask-free work; the diagonal batch (with its DVE mask adds) is last.
    """
    kpc = CH // KT  # k-tiles per chunk (4)
    batches = []
    if causal:
        nd = list(range(0, kpc * c))
    else:
        nd = list(range(0, S // KT))
    # split into groups of <=3 (psum budget), preferring even group sizes so
    # dual-tenant QK pairs never run unpaired
    if len(nd) % 3 == 1 and len(nd) >= 4:
        sizes = [3] * (len(nd) // 3 - 1) + [2, 2]
    else:
        sizes = [3] * (len(nd) // 3) + ([len(nd) % 3] if len(nd) % 3 else [])
    g = 0
    for sz in sizes:
        grp = nd[g : g + sz]
        g += sz
        batches.append(
            (512 * len(grp), [(j, i * 512, 512, 0, False) for i, j in enumerate(grp)])
        )
    if causal:
        # diagonal k-tiles j=kpc*c+r; packed order r0,r1,r3,r2 fills
        # [0,1280) with every matmul within a psum bank
        d0 = kpc * c
        diag = [
            (d0 + 0, 0, 512, 0, True),
            (d0 + 1, 512, 384, 128, True),
            (d0 + 3, 896, 128, 384, True),
            (d0 + 2, 1024, 256, 256, True),
        ]
        batches.append((1280, diag))
    return batches


def _build(causal):
    nc = bacc.Bacc(None, target_bir_lowering=False)
    # All DRAM I/O is f32-typed (bf16 host arrays hang the axon transport);
    # qt/kt/va carry bf16 PAIRS packed into f32 words, unpacked on device
    # for free via AP.bitcast views.  Big contiguous descriptors only.
    njt = S // KT  # k-tiles per head
    VW = D + 1  # V columns incl. the baked-in ones column
    qt = nc.declare_dram_parameter("qt", [HPC, 2 * D, S // 2], F32, isOutput=False)
    kt = nc.declare_dram_parameter("kt", [HPC, 2 * D, S // 2], F32, isOutput=False)
    va = nc.declare_dram_parameter("va", [HPC, KT, njt * VW // 2], F32, isOutput=False)
    # cm: [128, 128+512] bf16 packed in f32 pairs - identity (cols 0:128)
    # then the additive causal mask (cols 128:640, NEG strictly above diag)
    cm = nc.declare_dram_parameter("cm", [KT, (KT + CH) // 2], F32, isOutput=False)
    o = nc.declare_dram_parameter("o", [HPC, VW, S], F32, isOutput=True)

    nchunks = S // CH

    with tile.TileContext(nc) as tc:
        with (
            tc.tile_pool(name="const", bufs=1) as const,
            tc.tile_pool(name="qk", bufs=2) as qk_pool,
            tc.tile_pool(name="vaug", bufs=2) as vaug_pool,
            tc.tile_pool(name="pt", bufs=4) as pt_pool,
            tc.tile_pool(name="osb", bufs=2) as osb_pool,
            tc.tile_pool(name="st", bufs=2, space="PSUM") as st_pool,
            tc.tile_pool(name="acc", bufs=2, space="PSUM") as acc_pool,
        ):
            cm_sb = const.tile([KT, KT + CH], BF16)
            nc.sync.dma_start(out=cm_sb.bitcast(F32), in_=cm[0:KT])
            ident = cm_sb[:, 0:KT]
            negmask = cm_sb[:, KT : KT + CH]

            # Input DMAs are issued one head ahead so the (program-order
            # earlier) output DMA of head h never blocks head h+1's loads
            # on the sync queue.  Head 0's q/k arrive in 512-col pieces so
            # the first QK starts after ~1/4 of the transfer.
            def load_head(h):
                qt_sb = qk_pool.tile([2 * D, S], BF16, tag="qt", name="qt_sb")
                kt_sb = qk_pool.tile([2 * D, S], BF16, tag="kt", name="kt_sb")
                v_aug = vaug_pool.tile(
                    [KT, njt * VW], BF16, tag="va", name="v_aug"
                )
                if h == 0:
                    qf = S // 8  # 512 bf16 cols = 256 packed f32 cols
                    for p in range(4):
                        nc.sync.dma_start(
                            out=qt_sb.bitcast(F32)[:, p * qf : (p + 1) * qf],
                            in_=qt[h][:, p * qf : (p + 1) * qf],
                        )
                        nc.sync.dma_start(
                            out=kt_sb.bitcast(F32)[:, p * qf : (p + 1) * qf],
                            in_=kt[h][:, p * qf : (p + 1) * qf],
                        )
                else:
                    nc.sync.dma_start(out=qt_sb.bitcast(F32), in_=qt[h])
                    nc.sync.dma_start(out=kt_sb.bitcast(F32), in_=kt[h])
                nc.sync.dma_start(out=v_aug.bitcast(F32), in_=va[h])
                return qt_sb, kt_sb, v_aug

            # One flat software pipeline across ALL heads: the pending PV
            # batch crosses head boundaries, so each head's first QK+mask
            # chain hides under the previous head's last ACTIVATE.
            def emit_pv(item):
                (c, first, last, blocks, pt, acc, v_aug_i, o_sb_i, odma) = item
                n = len(blocks)
                for i, (j, off, span, qlo, diag) in enumerate(blocks):
                    jc = j * VW
                    nc.tensor.matmul(
                        acc[:, qlo : qlo + span],
                        lhsT=v_aug_i[0:KT, jc : jc + VW],
                        rhs=pt[0:KT, off : off + span],
                        start=(first and i == 0),
                        stop=(last and i == n - 1),
                    )
                if last:
                    nc.vector.tensor_copy(
                        o_sb_i[:, c * CH : (c + 1) * CH], acc
                    )
                    if odma is not None:
                        nc.sync.dma_start(
                            out=odma[:, c * CH : (c + 1) * CH],
                            in_=o_sb_i[:, c * CH : (c + 1) * CH],
                        )

            pending = None
            qk_parity = 0
            nxt = load_head(0)
            for h in range(HPC):
                qt_sb, kt_sb, v_aug = nxt
                if h + 1 < HPC:
                    nxt = load_head(h + 1)

                o_sb = osb_pool.tile([VW, S], F32)

                # Flatten all (chunk, batch) work items for this head.
                work = []  # (c, acc_first, acc_last, bw, blocks)
                for c in range(nchunks):
                    batches = _plan_chunk(c, causal)
                    for bi, (bw, blocks) in enumerate(batches):
                        work.append(
                            (c, bi == 0, bi == len(batches) - 1, bw, blocks)
                        )

                accs = {}  # chunk -> acc tile

                for item in work:
                    c, first, last, bw, blocks = item
                    if first:
                        accs[c] = acc_pool.tile(
                            [VW, CH], F32, tag="acc", name="acc"
                        )
                    st = st_pool.tile([KT, 1536], F32, tag="st")
                    # start=True clears has_written for the WHOLE psum bank,
                    # so only the first matmul touching each bank may carry
                    # it: a later start=True would wipe a sibling block's
                    # bits and turn its pending mask-accumulate into an
                    # overwrite.  start=False on a bank-cleared region still
                    # overwrites (per-element: add where bit set, write
                    # where clear), which is exactly what block r3 needs.
                    started_banks = set()
                    for j, off, span, qlo, diag in blocks:
                        p0 = D * qk_parity  # row-group tenant 0 or 64
                        qk_parity ^= 1
                        bank = off // 512
                        nc.tensor.matmul(
                            st[:, off : off + span],
                            lhsT=kt_sb[p0 : p0 + D, j * KT : (j + 1) * KT],
                            rhs=qt_sb[
                                p0 : p0 + D,
                                c * CH + qlo : c * CH + qlo + span,
                            ],
                            start=bank not in started_banks,
                            stop=not diag,
                        )
                        started_banks.add(bank)
                    # causal mask applied by the PE itself: st += I.T @
                    # negmask (keeps the QK->exp chain off the DVE, whose
                    # psum access serializes against matmuls bank-by-bank)
                    for j, off, span, qlo, diag in blocks:
                        if diag:
                            nc.tensor.matmul(
                                st[:, off : off + span],
                                lhsT=ident,
                                rhs=negmask[:, 0:span],
                                start=False,
                                stop=True,
                            )
                    pt = pt_pool.tile([KT, 1536], BF16, tag="pt")
                    nc.scalar.activation(
                        pt[:, :bw],
                        st[:, :bw],
                        mybir.ActivationFunctionType.Exp,
                        scale=float(1.0 / np.sqrt(D)),
                    )
                    if pending is not None:
                        emit_pv(pending)
                    pending = (
                        c, first, last, blocks, pt,
                        accs[c], v_aug, o_sb, o[h],
                    )
            if pending is not None:
                emit_pv(pending)
    nc.compile()
    return nc


_CACHE = {}


def _get_nc(causal):
    if causal not in _CACHE:
        _CACHE[causal] = _build(causal)
    return _CACHE[causal]


def _prep_inputs(q, k, v):
    """Shard + pre-transpose + bf16-pack on host -> per-core in_maps.

    qt/kt: head-major [BH, D, S] bf16, adjacent pairs packed into f32.
    va: v_aug [BH, 128, njt*65] bf16 (v tiles k-major on partitions with a
    ones column per tile), packed into f32 the same way.
    """
    import ml_dtypes

    njt = S // KT
    VW = D + 1
    q = np.asarray(q, dtype=np.float32).reshape(B * H, S, D)
    k = np.asarray(k, dtype=np.float32).reshape(B * H, S, D)
    v = np.asarray(v, dtype=np.float32).reshape(B * H, S, D)
    qt1 = np.ascontiguousarray(q.transpose(0, 2, 1)).astype(ml_dtypes.bfloat16)
    kt1 = np.ascontiguousarray(k.transpose(0, 2, 1)).astype(ml_dtypes.bfloat16)
    # duplicate on partitions 64..127 for the second row-group tenant
    qt = np.concatenate([qt1, qt1], axis=1)  # [BH, 2D, S]
    kt = np.concatenate([kt1, kt1], axis=1)
    va = np.empty((B * H, KT, njt, VW), dtype=ml_dtypes.bfloat16)
    va[..., :D] = v.reshape(B * H, njt, KT, D).transpose(0, 2, 1, 3)
    va[..., D] = 1.0
    qt_p = qt.view(np.float32)  # [BH, 2D, S//2]
    kt_p = kt.view(np.float32)
    va_p = va.reshape(B * H, KT, njt * VW).view(np.float32)
    # identity + additive causal mask, streamed through the PE on device
    cmh = np.zeros((KT, KT + CH), dtype=ml_dtypes.bfloat16)
    cmh[:, :KT] = np.eye(KT, dtype=np.float32)
    i_idx = np.arange(KT)[:, None]
    j_idx = np.arange(CH)[None, :]
    cmh[:, KT:] = np.where(j_idx >= i_idx, 0.0, NEG).astype(ml_dtypes.bfloat16)
    cm_p = np.ascontiguousarray(cmh.view(np.float32))
    in_maps = []
    for i in range(N_CORES):
        sl = slice(i * HPC, (i + 1) * HPC)
        in_maps.append(
            {
                "qt": np.ascontiguousarray(qt_p[sl]),
                "kt": np.ascontiguousarray(kt_p[sl]),
                "va": np.ascontiguousarray(va_p[sl]),
                "cm": cm_p,
            }
        )
    return in_maps


def _postprocess(results):
    """Per-core [HPC, D+1, S] -> full [B, H, S, D] (divide + transpose)."""
    outs = []
    for i in range(N_CORES):
        oc = results[i]["o"]  # [HPC, D+1, S]
        num = oc[:, :D, :]  # [HPC, D, S]
        den = oc[:, D : D + 1, :]  # [HPC, 1, S]
        outs.append((num / den).transpose(0, 2, 1))  # [HPC, S, D]
    return np.concatenate(outs, axis=0).reshape(B, H, S, D).astype(np.float32)


def _run(q, k, v, mask, trace=False):
    mask = np.asarray(mask)
    causal = bool(np.array_equal(mask, np.tril(np.ones((S, S), dtype=bool))))
    if not causal:
        assert mask.all(), (
            "only causal (tril) or all-ones masks are supported by this kernel"
        )
    nc = _get_nc(causal)
    in_maps = _prep_inputs(q, k, v)
    res = run_bass_kernel_spmd(nc, in_maps, list(range(N_CORES)), trace=trace)
    out = _postprocess(res.results)
    return out, res


def kernel(q, k, v, mask):
    out, _ = _run(q, k, v, mask, trace=False)
    return out


# revision 20
# speedup vs baseline: 2.2887x; 1.0055x over previous
"""Causal multi-head attention on 8 Trainium2 NeuronCores (Bass/Tile).

Problem: B=4 H=16 S=2048 D=64 fp32, causal mask, softmax(QK^T/sqrt(D))V.
Sharding: batch*heads (64) split 8 per core; no cross-core communication.

Design notes (v2)
-----------------
- Host pre-transposes Q,K to [d, s] per head so the device needs zero
  transposes; scores are computed TRANSPOSED (S^T[k, q]) so softmax's
  P^T is directly the moving operand of the P@V matmul.
- Softmax over k (= partition dim in S^T) avoids max-subtraction (scores
  ~N(0,1) after 1/sqrt(64) scaling) and gets the denominator free via a
  ones-column appended to V.  Final divide + transpose happen on host.
- QK matmuls contract over d=64 and run as two concurrent row-group
  tenants (Q/K duplicated on partitions 64..127) -> ~2 cols/cycle.
- PV runs single-tenant K=128 into ONE psum bank per chunk (acc pool
  bufs=2 double-buffers across chunks); the old dual-tenant accA/accB +
  DVE merge is gone - one DVE copy psum->sbuf per chunk remains.
- Causal masking: only the [128,128] diagonal square of each diagonal
  block differs from zero, so the DVE additive mask covers 128 cols per
  diag tile instead of the full span (2.6x less DVE work), off the
  scalar engine's critical path.
- Emission is software-pipelined: each batch's PV is emitted AFTER the
  next batch's QK+exp, so the scalar engine (the throughput floor at
  ~1 elem/lane/cycle for exp) stays saturated and the PE never waits
  on an ACTIVATE it just enqueued.
- All matmuls bf16 (fp32 PE matmuls stream multi-pass, ~3x slower);
  fp32 accumulation in PSUM; exp computed in fp32 from PSUM.
"""

import os
import sys

import numpy as np

sys.path.insert(0, "/opt/trn_rl_repo")

import concourse.bass as bass  # noqa: E402
import concourse.tile as tile  # noqa: E402
from concourse import bacc, mybir  # noqa: E402
from concourse.bass_utils import run_bass_kernel_spmd  # noqa: E402

B, H, S, D = 4, 16, 2048, 64
N_CORES = 8
HPC = (B * H) // N_CORES  # heads per core
KT = 128   # k-tile rows
CH = 512   # q-chunk cols
NEG = -1e9

F32 = mybir.dt.float32
BF16 = mybir.dt.bfloat16


def _plan_chunk(c, causal):
    """Per q-chunk list of ACTIVATE batches.

    Each batch is (width, [(j, off, span, qlo, diag), ...]): k-tile j's
    scores for q-columns [qlo, qlo+span) of the chunk land at packed psum
    columns [off, off+span).  Offsets never let a matmul cross a 512-col
    psum bank boundary.  `diag` marks blocks needing the causal mask.
    Non-diagonal batches come first so each chunk's pipeline starts with
    mask-free work; the diagonal batch (with its DVE mask adds) is last.
    """
    kpc = CH // KT  # k-tiles per chunk (4)
    batches = []
    if causal:
        nd = list(range(0, kpc * c))
    else:
        nd = list(range(0, S // KT))
    # split into groups of <=3 (psum budget), preferring even group sizes so
    # dual-tenant QK pairs never run unpaired
    if len(nd) % 3 == 1 and len(nd) >= 4:
        sizes = [3] * (len(nd) // 3 - 1) + [2, 2]
    else:
        sizes = [3] * (len(nd) // 3) + ([len(nd) % 3] if len(nd) % 3 else [])
    g = 0
    for sz in sizes:
        grp = nd[g : g + sz]
        g += sz
        batches.append(
            (512 * len(grp), [(j, i * 512, 512, 0, False) for i, j in enumerate(grp)])
        )
    if causal:
        # diagonal k-tiles j=kpc*c+r; packed order r0,r1,r3,r2 fills
        # [0,1280) with every matmul within a psum bank
        d0 = kpc * c
        diag = [
            (d0 + 0, 0, 512, 0, True),
            (d0 + 1, 512, 384, 128, True),
            (d0 + 3, 896, 128, 384, True),
            (d0 + 2, 1024, 256, 256, True),
        ]
        batches.append((1280, diag))
    return batches


def _build(causal):
    nc = bacc.Bacc(None, target_bir_lowering=False)
    # All DRAM I/O is f32-typed (bf16 host arrays hang the axon transport);
    # qt/kt/va carry bf16 PAIRS packed into f32 words, unpacked on device
    # for free via AP.bitcast views.  Big contiguous descriptors only.
    njt = S // KT  # k-tiles per head
    VW = D + 1  # V columns incl. the baked-in ones column
    qt = nc.declare_dram_parameter("qt", [HPC, 2 * D, S // 2], F32, isOutput=False)
    kt = nc.declare_dram_parameter("kt", [HPC, 2 * D, S // 2], F32, isOutput=False)
    va = nc.declare_dram_parameter("va", [HPC, KT, njt * VW // 2], F32, isOutput=False)
    # cm: [128, 128+1280] bf16 packed in f32 pairs - identity (cols 0:128)
    # then the additive causal mask pre-packed in the diagonal-batch psum
    # layout (cols 128:1408): bank-aligned segments for r0|r1|r3|r2
    cm = nc.declare_dram_parameter(
        "cm", [KT, (KT + 1280) // 2], F32, isOutput=False
    )
    o = nc.declare_dram_parameter("o", [HPC, VW, S], F32, isOutput=True)

    nchunks = S // CH

    with tile.TileContext(nc) as tc:
        with (
            tc.tile_pool(name="const", bufs=1) as const,
            tc.tile_pool(name="qk", bufs=2) as qk_pool,
            tc.tile_pool(name="vaug", bufs=2) as vaug_pool,
            tc.tile_pool(name="pt", bufs=4) as pt_pool,
            tc.tile_pool(name="osb", bufs=2) as osb_pool,
            tc.tile_pool(name="st", bufs=2, space="PSUM") as st_pool,
            tc.tile_pool(name="acc", bufs=2, space="PSUM") as acc_pool,
        ):
            cm_sb = const.tile([KT, KT + 1280], BF16)
            nc.sync.dma_start(out=cm_sb.bitcast(F32), in_=cm[0:KT])
            ident = cm_sb[:, 0:KT]
            negpack = cm_sb[:, KT : KT + 1280]

            # Input DMAs are issued one head ahead so the (program-order
            # earlier) output DMA of head h never blocks head h+1's loads
            # on the sync queue.  Head 0's q/k arrive in 512-col pieces so
            # the first QK starts after ~1/4 of the transfer.
            def load_head(h):
                qt_sb = qk_pool.tile([2 * D, S], BF16, tag="qt", name="qt_sb")
                kt_sb = qk_pool.tile([2 * D, S], BF16, tag="kt", name="kt_sb")
                v_aug = vaug_pool.tile(
                    [KT, njt * VW], BF16, tag="va", name="v_aug"
                )
                if h == 0:
                    qf = S // 8  # 512 bf16 cols = 256 packed f32 cols
                    for p in range(4):
                        nc.sync.dma_start(
                            out=qt_sb.bitcast(F32)[:, p * qf : (p + 1) * qf],
                            in_=qt[h][:, p * qf : (p + 1) * qf],
                        )
                        nc.sync.dma_start(
                            out=kt_sb.bitcast(F32)[:, p * qf : (p + 1) * qf],
                            in_=kt[h][:, p * qf : (p + 1) * qf],
                        )
                else:
                    nc.sync.dma_start(out=qt_sb.bitcast(F32), in_=qt[h])
                    nc.sync.dma_start(out=kt_sb.bitcast(F32), in_=kt[h])
                nc.sync.dma_start(out=v_aug.bitcast(F32), in_=va[h])
                return qt_sb, kt_sb, v_aug

            # One flat software pipeline across ALL heads: the pending PV
            # batch crosses head boundaries, so each head's first QK+mask
            # chain hides under the previous head's last ACTIVATE.
            def emit_pv(item):
                (c, first, last, blocks, pt, acc, v_aug_i, o_sb_i, odma) = item
                n = len(blocks)
                for i, (j, off, span, qlo, diag) in enumerate(blocks):
                    jc = j * VW
                    nc.tensor.matmul(
                        acc[:, qlo : qlo + span],
                        lhsT=v_aug_i[0:KT, jc : jc + VW],
                        rhs=pt[0:KT, off : off + span],
                        start=(first and i == 0),
                        stop=(last and i == n - 1),
                    )
                if last:
                    nc.vector.tensor_copy(
                        o_sb_i[:, c * CH : (c + 1) * CH], acc
                    )
                    if odma is not None:
                        nc.sync.dma_start(
                            out=odma[:, c * CH : (c + 1) * CH],
                            in_=o_sb_i[:, c * CH : (c + 1) * CH],
                        )

            pending = None
            qk_parity = 0
            nxt = load_head(0)
            for h in range(HPC):
                qt_sb, kt_sb, v_aug = nxt
                if h + 1 < HPC:
                    nxt = load_head(h + 1)

                o_sb = osb_pool.tile([VW, S], F32)

                # Flatten all (chunk, batch) work items for this head.
                # Chunk 0 (mask-heavy, single diag batch) goes last so the
                # head-boundary batch is a cheap mask-free QK group.
                work = []  # (c, acc_first, acc_last, bw, blocks)
                corder = [1, 2, 3, 0] if causal else range(nchunks)
                for c in corder:
                    batches = _plan_chunk(c, causal)
                    for bi, (bw, blocks) in enumerate(batches):
                        work.append(
                            (c, bi == 0, bi == len(batches) - 1, bw, blocks)
                        )

                accs = {}  # chunk -> acc tile

                for item in work:
                    c, first, last, bw, blocks = item
                    if first:
                        accs[c] = acc_pool.tile(
                            [VW, CH], F32, tag="acc", name="acc"
                        )
                    st = st_pool.tile([KT, 1536], F32, tag="st")
                    is_diag = blocks[0][4]
                    if is_diag:
                        # Causal mask FIRST, via the PE (st = I.T @ negpack,
                        # one matmul per psum bank, start=True clears the
                        # bank); the QK matmuls then ACCUMULATE onto it
                        # (start=False).  This keeps the masks off the
                        # QK->exp critical chain - they run early, hidden
                        # under the previous ACTIVATE - and off the DVE,
                        # whose psum access serializes against matmuls.
                        for mo, mw in ((0, 512), (512, 512), (1024, 256)):
                            nc.tensor.matmul(
                                st[:, mo : mo + mw],
                                lhsT=ident,
                                rhs=negpack[:, mo : mo + mw],
                                start=True,
                                stop=False,
                            )
                    for j, off, span, qlo, diag in blocks:
                        p0 = D * qk_parity  # row-group tenant 0 or 64
                        qk_parity ^= 1
                        nc.tensor.matmul(
                            st[:, off : off + span],
                            lhsT=kt_sb[p0 : p0 + D, j * KT : (j + 1) * KT],
                            rhs=qt_sb[
                                p0 : p0 + D,
                                c * CH + qlo : c * CH + qlo + span,
                            ],
                            start=not diag,
                            stop=True,
                        )
                    pt = pt_pool.tile([KT, 1536], BF16, tag="pt")
                    nc.scalar.activation(
                        pt[:, :bw],
                        st[:, :bw],
                        mybir.ActivationFunctionType.Exp,
                        scale=float(1.0 / np.sqrt(D)),
                    )
                    if pending is not None:
                        emit_pv(pending)
                    pending = (
                        c, first, last, blocks, pt,
                        accs[c], v_aug, o_sb, o[h],
                    )
            if pending is not None:
                emit_pv(pending)
    nc.compile()
    return nc


_CACHE = {}


def _get_nc(causal):
    if causal not in _CACHE:
        _CACHE[causal] = _build(causal)
    return _CACHE[causal]


def _prep_inputs(q, k, v):
    """Shard + pre-transpose + bf16-pack on host -> per-core in_maps.

    qt/kt: head-major [BH, D, S] bf16, adjacent pairs packed into f32.
    va: v_aug [BH, 128, njt*65] bf16 (v tiles k-major on partitions with a
    ones column per tile), packed into f32 the same way.
    """
    import ml_dtypes

    njt = S // KT
    VW = D + 1
    q = np.asarray(q, dtype=np.float32).reshape(B * H, S, D)
    k = np.asarray(k, dtype=np.float32).reshape(B * H, S, D)
    v = np.asarray(v, dtype=np.float32).reshape(B * H, S, D)
    qt1 = np.ascontiguousarray(q.transpose(0, 2, 1)).astype(ml_dtypes.bfloat16)
    kt1 = np.ascontiguousarray(k.transpose(0, 2, 1)).astype(ml_dtypes.bfloat16)
    # duplicate on partitions 64..127 for the second row-group tenant
    qt = np.concatenate([qt1, qt1], axis=1)  # [BH, 2D, S]
    kt = np.concatenate([kt1, kt1], axis=1)
    va = np.empty((B * H, KT, njt, VW), dtype=ml_dtypes.bfloat16)
    va[..., :D] = v.reshape(B * H, njt, KT, D).transpose(0, 2, 1, 3)
    va[..., D] = 1.0
    qt_p = qt.view(np.float32)  # [BH, 2D, S//2]
    kt_p = kt.view(np.float32)
    va_p = va.reshape(B * H, KT, njt * VW).view(np.float32)
    # identity + additive causal mask, streamed through the PE on device.
    # The mask is pre-packed in the diagonal-batch psum layout (bank-
    # aligned segments r0|r1|r3|r2 at offsets 0/512/896/1024).
    cmh = np.zeros((KT, KT + 1280), dtype=ml_dtypes.bfloat16)
    cmh[:, :KT] = np.eye(KT, dtype=np.float32)
    i_idx = np.arange(KT)[:, None]
    j_idx = np.arange(CH)[None, :]
    m = np.where(j_idx >= i_idx, 0.0, NEG).astype(ml_dtypes.bfloat16)
    for off, span in ((0, 512), (512, 384), (896, 128), (1024, 256)):
        cmh[:, KT + off : KT + off + span] = m[:, :span]
    cm_p = np.ascontiguousarray(cmh.view(np.float32))
    in_maps = []
    for i in range(N_CORES):
        sl = slice(i * HPC, (i + 1) * HPC)
        in_maps.append(
            {
                "qt": np.ascontiguousarray(qt_p[sl]),
                "kt": np.ascontiguousarray(kt_p[sl]),
                "va": np.ascontiguousarray(va_p[sl]),
                "cm": cm_p,
            }
        )
    return in_maps


def _postprocess(results):
    """Per-core [HPC, D+1, S] -> full [B, H, S, D] (divide + transpose)."""
    outs = []
    for i in range(N_CORES):
        oc = results[i]["o"]  # [HPC, D+1, S]
        num = oc[:, :D, :]  # [HPC, D, S]
        den = oc[:, D : D + 1, :]  # [HPC, 1, S]
        outs.append((num / den).transpose(0, 2, 1))  # [HPC, S, D]
    return np.concatenate(outs, axis=0).reshape(B, H, S, D).astype(np.float32)


def _run(q, k, v, mask, trace=False):
    mask = np.asarray(mask)
    causal = bool(np.array_equal(mask, np.tril(np.ones((S, S), dtype=bool))))
    if not causal:
        assert mask.all(), (
            "only causal (tril) or all-ones masks are supported by this kernel"
        )
    nc = _get_nc(causal)
    in_maps = _prep_inputs(q, k, v)
    res = run_bass_kernel_spmd(nc, in_maps, list(range(N_CORES)), trace=trace)
    out = _postprocess(res.results)
    return out, res


def kernel(q, k, v, mask):
    out, _ = _run(q, k, v, mask, trace=False)
    return out
